# revision 1
# baseline (speedup 1.0000x reference)
"""ALIGN module kernel for 8 TRN2 NeuronCores (vocab-parallel).

Reference computation (B=4, S=576, Dv=1024, Dl=4096, V=32000):
    x  = vision_feats @ W1_w.T + W1_b          # [T=2304, Dl]
    xn = layernorm(x)                          # over Dl, no affine
    P  = softmax(xn @ W2_w.T, axis=-1)         # [T, V]
    F  = P @ llm_token_embed                   # [T, Dl]

Sharding: vocab dim of W2_w / llm_token_embed split across the 8 cores
(4000 rows each, zero-padded to 4096). Stage A (W1 + LN) is token-parallel
(288 tokens/core) followed by an AllGather of xn (bf16). Softmax needs no
max-subtraction (logits are ~N(0,1), |logit| < ~6): each core computes
exp(logits_loc), the pad rows contribute exactly exp(0)=1 each so the local
denominator is corrected by -96, denominators are AllReduced, numerators are
divided locally then ReduceScattered (division commutes with the sum).

Host-side prep encodes all layout work: weights arrive pre-transposed,
pre-padded and pre-cast to bf16 so the device never transposes W1/W2.
"""

import os
import sys

for _p in ("/opt/trn_rl_repo", "/root/.axon_site/_ro/trn_rl_repo"):
    if os.path.isdir(_p) and _p not in sys.path:
        sys.path.insert(0, _p)

import numpy as np
import ml_dtypes

from concourse import bass, bacc, mybir, tile
from concourse.bass_utils import run_bass_kernel_spmd
from concourse.masks import make_identity

BF16NP = ml_dtypes.bfloat16
F32 = mybir.dt.float32
BF16 = mybir.dt.bfloat16

N_CORES = 8
T = 2304          # total tokens (B*S)
T_LOC = 288       # tokens per core in stage A
DV = 1024
DL = 4096
V_PAD = 4096      # padded vocab rows per core (4000 real + 96 zero pads)
N_PAD = 96.0

N_SB = 3          # token superblocks in phase B
TSB = 768         # tokens per superblock
N_TT = 6          # 128-row token tiles per superblock
C1 = 384          # matmul1 token-chunk width (2 chunks per superblock)
EC = 256          # matmul2 embedding-chunk width
N_EC = DL // EC   # 16 e-chunks
RS_ROWS = TSB // N_CORES  # 96 rows per core out of each ReduceScatter

_NC_CACHE = None


def build():
    nc = bacc.Bacc("TRN2", target_bir_lowering=False, debug=False,
                   num_devices=N_CORES)
    rg = [list(range(N_CORES))]

    visionT = nc.dram_tensor("visionT", [DV, T_LOC], BF16, kind="ExternalInput")
    w1t = nc.dram_tensor("w1t", [DV, DL], BF16, kind="ExternalInput")
    w1b = nc.dram_tensor("w1b", [1, DL], F32, kind="ExternalInput")
    w2t = nc.dram_tensor("w2t", [V_PAD // 128, DL, 128], BF16,
                         kind="ExternalInput")
    emb = nc.dram_tensor("emb", [V_PAD, DL], BF16, kind="ExternalInput")
    out = nc.dram_tensor("out", [N_SB, RS_ROWS, DL], F32, kind="ExternalOutput")

    with tile.TileContext(nc) as tc:
        with tc.tile_pool(name="consts", bufs=1) as consts, \
             tc.tile_pool(name="dram", bufs=1, space="DRAM") as dram, \
             tc.tile_pool(name="dram_s", bufs=2, space="DRAM") as dram_s, \
             tc.tile_pool(name="dram_rs", bufs=3, space="DRAM") as dram_rs:

            ident = consts.tile([128, 128], BF16)
            make_identity(nc, ident)
            ones_col = consts.tile([128, 1], BF16)
            nc.vector.memset(ones_col, 1.0)
            eps_col = consts.tile([128, 1], F32)
            nc.vector.memset(eps_col, 1e-5)

            ag_in = dram.tile([T_LOC, DL], BF16)
            ag_out = dram.tile([T, DL], BF16, addr_space="Shared")

            # ---------------- Stage A: x = visionT.T @ W1T + b, LN, -> bf16
            with tc.tile_pool(name="stageA", bufs=1) as sa, \
                 tc.tile_pool(name="stageA2", bufs=2) as sa2, \
                 tc.tile_pool(name="psumA", bufs=2, space="PSUM") as psa:
                vt_sb = sa.tile([128, DV // 128, T_LOC], BF16)
                nc.sync.dma_start(
                    vt_sb, visionT[:].rearrange("(k p) t -> p k t", p=128))
                w1t_sb = sa.tile([128, DV // 128, DL], BF16)
                nc.sync.dma_start(
                    w1t_sb, w1t[:].rearrange("(k p) d -> p k d", p=128))
                bias_bc = sa.tile([128, DL], F32)
                nc.sync.dma_start(
                    bias_bc,
                    bass.AP(tensor=w1b, offset=0, ap=[[0, 128], [1, DL]]))

                t_sizes = [128, 128, 32]
                for a in range(3):
                    ta = t_sizes[a]
                    t0 = 128 * a
                    x_sb = sa2.tile([128, DL], F32, tag="x")
                    for n in range(DL // 512):
                        xp = psa.tile([128, 512], F32, tag="xp")
                        for k in range(DV // 128):
                            nc.tensor.matmul(
                                xp[:ta], lhsT=vt_sb[:, k, t0:t0 + ta],
                                rhs=w1t_sb[:, k, 512 * n:512 * (n + 1)],
                                start=(k == 0), stop=(k == DV // 128 - 1))
                        nc.vector.tensor_tensor(
                            out=x_sb[:ta, 512 * n:512 * (n + 1)],
                            in0=xp[:ta],
                            in1=bias_bc[:ta, 512 * n:512 * (n + 1)],
                            op=mybir.AluOpType.add)
                    # LayerNorm over DL
                    stats = sa2.tile([128, DL // 512, 6], F32, tag="stats")
                    for g in range(DL // 512):
                        nc.vector.bn_stats(
                            out=stats[:ta, g, :],
                            in_=x_sb[:ta, 512 * g:512 * (g + 1)])
                    mv = sa2.tile([128, 2], F32, tag="mv")
                    nc.vector.bn_aggr(out=mv[:ta], in_=stats[:ta])
                    sd = sa2.tile([128, 1], F32, tag="sd")
                    nc.scalar.activation(
                        out=sd[:ta], in_=mv[:ta, 1:2],
                        func=mybir.ActivationFunctionType.Sqrt,
                        bias=eps_col[:ta])
                    rstd = sa2.tile([128, 1], F32, tag="rstd")
                    nc.vector.reciprocal(out=rstd[:ta], in_=sd[:ta])
                    xn_bf = sa2.tile([128, DL], BF16, tag="xn")
                    nc.vector.tensor_scalar(
                        out=xn_bf[:ta], in0=x_sb[:ta],
                        scalar1=mv[:ta, 0:1], scalar2=rstd[:ta],
                        op0=mybir.AluOpType.subtract,
                        op1=mybir.AluOpType.mult)
                    nc.sync.dma_start(ag_in[t0:t0 + ta, :], xn_bf[:ta])

            nc.gpsimd.collective_compute(
                "AllGather", mybir.AluOpType.bypass, replica_groups=rg,
                ins=[ag_in.opt()], outs=[ag_out.opt()])

            # ---------------- Phase B
            with tc.tile_pool(name="xnt_p", bufs=1) as xnt_p, \
                 tc.tile_pool(name="pt_p", bufs=1) as pt_p, \
                 tc.tile_pool(name="xl_p", bufs=2) as xl_p, \
                 tc.tile_pool(name="w2_p", bufs=2) as w2_p, \
                 tc.tile_pool(name="eb_p", bufs=2) as eb_p, \
                 tc.tile_pool(name="fs_p", bufs=3) as fs_p, \
                 tc.tile_pool(name="small", bufs=2) as small, \
                 tc.tile_pool(name="tp_ps", bufs=2, space="PSUM") as tp_ps, \
                 tc.tile_pool(name="l_ps", bufs=2, space="PSUM") as l_ps, \
                 tc.tile_pool(name="s_ps", bufs=1, space="PSUM") as s_ps, \
                 tc.tile_pool(name="f_ps", bufs=2, space="PSUM") as f_ps:

                for sb in range(N_SB):
                    # transpose xn superblock -> xnt [d_local, j, t_local]
                    xnt = xnt_p.tile([128, DL // 128, TSB], BF16, tag="xnt")
                    for tt in range(N_TT):
                        xl = xl_p.tile([128, DL], BF16, tag="xl")
                        r0 = TSB * sb + 128 * tt
                        nc.sync.dma_start(xl, ag_out[r0:r0 + 128, :])
                        for j in range(DL // 128):
                            tp = tp_ps.tile([128, 128], BF16, tag="tp")
                            nc.tensor.transpose(
                                out=tp, in_=xl[:, 128 * j:128 * (j + 1)],
                                identity=ident)
                            nc.vector.tensor_copy(
                                out=xnt[:, j, 128 * tt:128 * (tt + 1)],
                                in_=tp)

                    # matmul1: logitsT per v-tile, exp -> pt, s accumulation
                    pt = pt_p.tile([128, V_PAD // 128, TSB], BF16, tag="pt")
                    sp0 = s_ps.tile([1, C1], F32, tag="sp0")
                    sp1 = s_ps.tile([1, C1], F32, tag="sp1")
                    sps = [sp0, sp1]
                    for vt in range(V_PAD // 128):
                        w2s = w2_p.tile([128, DL // 128, 128], BF16, tag="w2")
                        nc.sync.dma_start(
                            w2s, w2t[vt].rearrange("(j p) v -> p j v", p=128))
                        for c in range(2):
                            lp = l_ps.tile([128, C1], F32, tag="lp")
                            for j in range(DL // 128):
                                nc.tensor.matmul(
                                    lp, lhsT=w2s[:, j, :],
                                    rhs=xnt[:, j, C1 * c:C1 * (c + 1)],
                                    start=(j == 0), stop=(j == DL // 128 - 1))
                            nc.scalar.activation(
                                out=pt[:, vt, C1 * c:C1 * (c + 1)], in_=lp,
                                func=mybir.ActivationFunctionType.Exp)
                            nc.tensor.matmul(
                                sps[c], lhsT=ones_col,
                                rhs=pt[:, vt, C1 * c:C1 * (c + 1)],
                                start=(vt == 0), stop=(vt == V_PAD // 128 - 1))

                    # denominator: pad-correct, AllReduce, reciprocal
                    s_sb = small.tile([1, TSB], F32, tag="ssb")
                    for c in range(2):
                        nc.vector.tensor_scalar_add(
                            out=s_sb[0:1, C1 * c:C1 * (c + 1)], in0=sps[c],
                            scalar1=-N_PAD)
                    s_in = dram_s.tile([1, TSB], F32, tag="sin")
                    nc.sync.dma_start(s_in, s_sb)
                    s_out = dram_s.tile([1, TSB], F32, tag="sout",
                                        addr_space="Shared")
                    nc.gpsimd.collective_compute(
                        "AllReduce", mybir.AluOpType.add, replica_groups=rg,
                        ins=[s_in.opt()], outs=[s_out.opt()])
                    sg = small.tile([128, N_TT], F32, tag="sg")
                    nc.sync.dma_start(
                        sg,
                        bass.AP(tensor=s_out.tensor, offset=s_out.offset,
                                ap=[[1, 128], [128, N_TT]]))
                    rsg = small.tile([128, N_TT], F32, tag="rsg")
                    nc.vector.reciprocal(out=rsg, in_=sg)

                    # matmul2: F = (pt.T @ emb) / s, per e-chunk + RS
                    for e in range(N_EC):
                        eb = eb_p.tile([128, V_PAD // 128, EC], BF16, tag="eb")
                        nc.sync.dma_start(
                            eb,
                            emb[:, EC * e:EC * (e + 1)].rearrange(
                                "(vt p) n -> p vt n", p=128))
                        rs_in = dram_rs.tile([TSB, EC], F32, tag="rsin")
                        for tt in range(N_TT):
                            fp = f_ps.tile([128, EC], F32, tag="fp")
                            for vt in range(V_PAD // 128):
                                nc.tensor.matmul(
                                    fp, lhsT=pt[:, vt, 128 * tt:128 * (tt + 1)],
                                    rhs=eb[:, vt, :],
                                    start=(vt == 0),
                                    stop=(vt == V_PAD // 128 - 1))
                            fs = fs_p.tile([128, EC], F32, tag="fs")
                            nc.vector.tensor_scalar_mul(
                                out=fs, in0=fp, scalar1=rsg[:, tt:tt + 1])
                            nc.sync.dma_start(
                                rs_in[128 * tt:128 * (tt + 1), :], fs)
                        rs_out = dram_rs.tile([RS_ROWS, EC], F32, tag="rsout")
                        nc.gpsimd.collective_compute(
                            "ReduceScatter", mybir.AluOpType.add,
                            replica_groups=rg,
                            ins=[rs_in.opt()], outs=[rs_out.opt()])
                        nc.sync.dma_start(
                            out[sb, :, EC * e:EC * (e + 1)], rs_out[:])

    nc.compile()
    return nc


def _get_nc():
    global _NC_CACHE
    if _NC_CACHE is None:
        _NC_CACHE = build()
    return _NC_CACHE


def _prep_in_maps(vision_feats, W1_w, W1_b, W2_w, llm_token_embed):
    vf = np.ascontiguousarray(np.asarray(vision_feats, np.float32)).reshape(
        T, DV)
    W1 = np.asarray(W1_w, np.float32)
    b1 = np.ascontiguousarray(np.asarray(W1_b, np.float32)).reshape(1, DL)
    W2 = np.asarray(W2_w, np.float32)
    E = np.asarray(llm_token_embed, np.float32)

    w1t = np.ascontiguousarray(W1.T).astype(BF16NP)
    v_loc = 32000 // N_CORES
    in_maps = []
    for c in range(N_CORES):
        vT = np.ascontiguousarray(vf[T_LOC * c:T_LOC * (c + 1)].T).astype(
            BF16NP)
        w2p = np.zeros((V_PAD, DL), np.float32)
        w2p[:v_loc] = W2[v_loc * c:v_loc * (c + 1)]
        w2tt = np.ascontiguousarray(
            w2p.T.reshape(DL, V_PAD // 128, 128).transpose(1, 0, 2)).astype(
                BF16NP)
        ep = np.zeros((V_PAD, DL), np.float32)
        ep[:v_loc] = E[v_loc * c:v_loc * (c + 1)]
        in_maps.append({
            "visionT": vT,
            "w1t": w1t,
            "w1b": b1,
            "w2t": w2tt,
            "emb": ep.astype(BF16NP),
        })
    return in_maps


def run_on_cores(in_maps, trace=False, **kwargs):
    nc = _get_nc()
    return run_bass_kernel_spmd(nc, in_maps, core_ids=list(range(N_CORES)),
                                trace=trace, **kwargs)


def assemble(core_outs):
    full = np.empty((T, DL), np.float32)
    for c in range(N_CORES):
        o = np.asarray(core_outs[c])  # [N_SB, RS_ROWS, DL]
        for sb in range(N_SB):
            r0 = TSB * sb + RS_ROWS * c
            full[r0:r0 + RS_ROWS] = o[sb]
    return full.reshape(4, 576, DL)


def kernel(**inputs):
    in_maps = _prep_in_maps(**inputs)
    res = run_on_cores(in_maps)
    return assemble([r["out"] for r in res.results])


# revision 3
# speedup vs baseline: 1.0357x; 1.0357x over previous
"""ALIGN module kernel for 8 TRN2 NeuronCores (vocab-parallel).

Reference computation (B=4, S=576, Dv=1024, Dl=4096, V=32000):
    x  = vision_feats @ W1_w.T + W1_b          # [T=2304, Dl]
    xn = layernorm(x)                          # over Dl, no affine
    P  = softmax(xn @ W2_w.T, axis=-1)         # [T, V]
    F  = P @ llm_token_embed                   # [T, Dl]

Sharding: vocab dim of W2_w / llm_token_embed split across the 8 cores
(4000 rows each, zero-padded to 4096). Stage A (W1 + LN) is token-parallel
(288 tokens/core) followed by an AllGather of xn (bf16). Softmax needs no
max-subtraction (logits are ~N(0,1), |logit| < ~6): each core computes
exp(logits_loc); the 96 zero pad rows contribute exactly exp(0)=1 each, so
the local denominator is corrected by -96. Denominators are ReduceScattered
(each core only ever divides the 96-token slices it owns after the F
ReduceScatter, so no cross-core broadcast of s is needed at all).

Host-side prep encodes all layout work: weights arrive pre-transposed,
pre-padded, pre-tiled for unit-stride DMA, and pre-cast to bf16, so the
device never transposes W1/W2 and every big DMA runs at full burst size.
"""

import os
import sys

for _p in ("/opt/trn_rl_repo", "/root/.axon_site/_ro/trn_rl_repo"):
    if os.path.isdir(_p) and _p not in sys.path:
        sys.path.insert(0, _p)

import numpy as np
import ml_dtypes

from concourse import bass, bacc, mybir, tile
from concourse.bass_utils import run_bass_kernel_spmd
from concourse.masks import make_identity

BF16NP = ml_dtypes.bfloat16
F32 = mybir.dt.float32
BF16 = mybir.dt.bfloat16

N_CORES = 8
T = 2304          # total tokens (B*S)
T_LOC = 288       # tokens per core in stage A
DV = 1024
DL = 4096
V_PAD = 4096      # padded vocab rows per core (4000 real + 96 zero pads)
N_PAD = 96.0
NVT = V_PAD // 128  # 32 vocab tiles per core
NJ = DL // 128      # 32 contraction tiles

N_SB = 3          # token superblocks in phase B
TSB = 768         # tokens per superblock
N_TT = 6          # 128-row token tiles per superblock
C1 = 384          # matmul1 token-chunk width (2 chunks per superblock)
EC = 256          # matmul2 embedding-chunk width (SBUF tile)
N_EC = DL // EC   # 16 e-chunks
EG = 4            # e-chunks per ReduceScatter group
N_EG = N_EC // EG
EGW = EC * EG     # 1024 columns per RS
RS_ROWS = TSB // N_CORES  # 96 rows per core out of each ReduceScatter

_NC_CACHE = None


def build():
    nc = bacc.Bacc("TRN2", target_bir_lowering=False, debug=False,
                   num_devices=N_CORES)
    rg = [list(range(N_CORES))]

    visionT = nc.dram_tensor("visionT", [DV, T_LOC], BF16, kind="ExternalInput")
    w1t = nc.dram_tensor("w1t", [DV, DL], BF16, kind="ExternalInput")
    w1b = nc.dram_tensor("w1b", [1, DL], F32, kind="ExternalInput")
    # [vt][p][j][vi]: per-partition unit-stride 8KB runs
    w2t = nc.dram_tensor("w2t", [NVT, 128, NJ, 128], BF16, kind="ExternalInput")
    # [e][p][vt][n]: per-partition unit-stride 16KB runs
    emb = nc.dram_tensor("emb", [N_EC, 128, NVT, EC], BF16,
                         kind="ExternalInput")
    out = nc.dram_tensor("out", [N_SB, RS_ROWS, DL], F32, kind="ExternalOutput")

    from contextlib import ExitStack
    with tile.TileContext(nc) as tc, ExitStack() as ctx:
        consts = ctx.enter_context(tc.tile_pool(name="consts", bufs=1))
        dram = ctx.enter_context(tc.tile_pool(name="dram", bufs=1, space="DRAM"))
        dram_s = ctx.enter_context(tc.tile_pool(name="dram_s", bufs=2, space="DRAM"))
        dram_rs = ctx.enter_context(tc.tile_pool(name="dram_rs", bufs=3, space="DRAM"))
        if True:

            ident = consts.tile([128, 128], BF16)
            make_identity(nc, ident)
            ones_col = consts.tile([128, 1], BF16)
            nc.vector.memset(ones_col, 1.0)
            eps_col = consts.tile([128, 1], F32)
            nc.vector.memset(eps_col, 1e-5)

            ag_in = dram.tile([T_LOC, DL], BF16)
            ag_out = dram.tile([T, DL], BF16, addr_space="Shared")

            # ---------------- Stage A: x = visionT.T @ W1T + b, LN, -> bf16
            with ExitStack() as actx:
                sa = actx.enter_context(tc.tile_pool(name="stageA", bufs=1))
                sa2 = actx.enter_context(tc.tile_pool(name="stageA2", bufs=2))
                psa = actx.enter_context(tc.tile_pool(name="psumA", bufs=2, space="PSUM"))
                vt_sb = sa.tile([128, DV // 128, T_LOC], BF16)
                for k in range(DV // 128):
                    nc.sync.dma_start(
                        vt_sb[:, k, :], visionT[128 * k:128 * (k + 1), :])
                w1t_sb = sa.tile([128, DV // 128, DL], BF16)
                for k in range(DV // 128):
                    nc.sync.dma_start(
                        w1t_sb[:, k, :], w1t[128 * k:128 * (k + 1), :])
                bias_bc = sa.tile([128, DL], F32)
                nc.sync.dma_start(
                    bias_bc,
                    bass.AP(tensor=w1b, offset=0, ap=[[0, 128], [1, DL]]))

                t_sizes = [128, 128, 32]
                for a in range(3):
                    ta = t_sizes[a]
                    t0 = 128 * a
                    x_sb = sa2.tile([128, DL], F32, tag="x")
                    for n in range(DL // 512):
                        xp = psa.tile([128, 512], F32, tag="xp")
                        for k in range(DV // 128):
                            nc.tensor.matmul(
                                xp[:ta], lhsT=vt_sb[:, k, t0:t0 + ta],
                                rhs=w1t_sb[:, k, 512 * n:512 * (n + 1)],
                                start=(k == 0), stop=(k == DV // 128 - 1))
                        nc.vector.tensor_tensor(
                            out=x_sb[:ta, 512 * n:512 * (n + 1)],
                            in0=xp[:ta],
                            in1=bias_bc[:ta, 512 * n:512 * (n + 1)],
                            op=mybir.AluOpType.add)
                    # LayerNorm over DL
                    stats = sa2.tile([128, DL // 512, 6], F32, tag="stats")
                    for g in range(DL // 512):
                        nc.vector.bn_stats(
                            out=stats[:ta, g, :],
                            in_=x_sb[:ta, 512 * g:512 * (g + 1)])
                    mv = sa2.tile([128, 2], F32, tag="mv")
                    nc.vector.bn_aggr(out=mv[:ta], in_=stats[:ta])
                    sd = sa2.tile([128, 1], F32, tag="sd")
                    nc.scalar.activation(
                        out=sd[:ta], in_=mv[:ta, 1:2],
                        func=mybir.ActivationFunctionType.Sqrt,
                        bias=eps_col[:ta])
                    rstd = sa2.tile([128, 1], F32, tag="rstd")
                    nc.vector.reciprocal(out=rstd[:ta], in_=sd[:ta])
                    xn_bf = sa2.tile([128, DL], BF16, tag="xn")
                    nc.vector.tensor_scalar(
                        out=xn_bf[:ta], in0=x_sb[:ta],
                        scalar1=mv[:ta, 0:1], scalar2=rstd[:ta],
                        op0=mybir.AluOpType.subtract,
                        op1=mybir.AluOpType.mult)
                    nc.sync.dma_start(ag_in[t0:t0 + ta, :], xn_bf[:ta])

            nc.gpsimd.collective_compute(
                "AllGather", mybir.AluOpType.bypass, replica_groups=rg,
                ins=[ag_in.opt()], outs=[ag_out.opt()])

            # ---------------- Phase B
            xnt_p = ctx.enter_context(tc.tile_pool(name="xnt_p", bufs=1))
            pt_p = ctx.enter_context(tc.tile_pool(name="pt_p", bufs=1))
            xl_p = ctx.enter_context(tc.tile_pool(name="xl_p", bufs=2))
            w2_p = ctx.enter_context(tc.tile_pool(name="w2_p", bufs=3))
            eb_p = ctx.enter_context(tc.tile_pool(name="eb_p", bufs=2))
            fs_p = ctx.enter_context(tc.tile_pool(name="fs_p", bufs=3))
            fo_p = ctx.enter_context(tc.tile_pool(name="fo_p", bufs=2))
            small = ctx.enter_context(tc.tile_pool(name="small", bufs=2))
            tp_ps = ctx.enter_context(tc.tile_pool(name="tp_ps", bufs=2, space="PSUM"))
            l_ps = ctx.enter_context(tc.tile_pool(name="l_ps", bufs=2, space="PSUM"))
            s_ps = ctx.enter_context(tc.tile_pool(name="s_ps", bufs=1, space="PSUM"))
            f_ps = ctx.enter_context(tc.tile_pool(name="f_ps", bufs=2, space="PSUM"))
            if True:

                for sb in range(N_SB):
                    # transpose xn superblock -> xnt [d_local, j, t_local]
                    xnt = xnt_p.tile([128, NJ, TSB], BF16, tag="xnt")
                    for tt in range(N_TT):
                        xl = xl_p.tile([128, DL], BF16, tag="xl")
                        r0 = TSB * sb + 128 * tt
                        nc.sync.dma_start(xl, ag_out[r0:r0 + 128, :])
                        for j in range(NJ):
                            tp = tp_ps.tile([128, 128], BF16, tag="tp")
                            nc.tensor.transpose(
                                out=tp, in_=xl[:, 128 * j:128 * (j + 1)],
                                identity=ident)
                            nc.vector.tensor_copy(
                                out=xnt[:, j, 128 * tt:128 * (tt + 1)],
                                in_=tp)

                    # matmul1: logitsT per v-tile, exp -> pt
                    pt = pt_p.tile([128, NVT, TSB], BF16, tag="pt")
                    for vt in range(NVT):
                        w2s = w2_p.tile([128, NJ, 128], BF16, tag="w2")
                        nc.sync.dma_start(w2s, w2t[vt])
                        for c in range(2):
                            lp = l_ps.tile([128, C1], F32, tag="lp")
                            for j in range(NJ):
                                nc.tensor.matmul(
                                    lp, lhsT=w2s[:, j, :],
                                    rhs=xnt[:, j, C1 * c:C1 * (c + 1)],
                                    start=(j == 0), stop=(j == NJ - 1))
                            nc.scalar.activation(
                                out=pt[:, vt, C1 * c:C1 * (c + 1)], in_=lp,
                                func=mybir.ActivationFunctionType.Exp)

                    # denominator: s = sum_v pt - 96, ReduceScatter to own rows
                    sps = []
                    for c in range(2):
                        sp = s_ps.tile([1, C1], F32, tag=f"sp{c}")
                        for vt in range(NVT):
                            nc.tensor.matmul(
                                sp, lhsT=ones_col,
                                rhs=pt[:, vt, C1 * c:C1 * (c + 1)],
                                start=(vt == 0), stop=(vt == NVT - 1))
                        sps.append(sp)
                    s_sb = small.tile([1, TSB], F32, tag="ssb")
                    for c in range(2):
                        nc.vector.tensor_scalar_add(
                            out=s_sb[0:1, C1 * c:C1 * (c + 1)], in0=sps[c],
                            scalar1=-N_PAD)
                    s_in = dram_s.tile([1, TSB], F32, tag="sin")
                    nc.sync.dma_start(s_in, s_sb)
                    s_out = dram_s.tile([1, RS_ROWS], F32, tag="sout")
                    nc.gpsimd.collective_compute(
                        "ReduceScatter", mybir.AluOpType.add, replica_groups=rg,
                        ins=[s_in.opt()], outs=[s_out.opt()])
                    sg = small.tile([RS_ROWS, 1], F32, tag="sg")
                    nc.sync.dma_start(
                        sg,
                        bass.AP(tensor=s_out.tensor, offset=s_out.offset,
                                ap=[[1, RS_ROWS], [1, 1]]))
                    rsg = small.tile([RS_ROWS, 1], F32, tag="rsg")
                    nc.vector.reciprocal(out=rsg, in_=sg)

                    # matmul2: F_partial = pt.T @ emb, RS per e-group, then /s
                    for eg in range(N_EG):
                        rs_in = dram_rs.tile([TSB, EGW], F32, tag="rsin")
                        for ei in range(EG):
                            e = EG * eg + ei
                            eb = eb_p.tile([128, NVT, EC], BF16, tag="eb")
                            nc.sync.dma_start(eb, emb[e])
                            for tt in range(N_TT):
                                fp = f_ps.tile([128, EC], F32, tag="fp")
                                for vt in range(NVT):
                                    nc.tensor.matmul(
                                        fp,
                                        lhsT=pt[:, vt, 128 * tt:128 * (tt + 1)],
                                        rhs=eb[:, vt, :],
                                        start=(vt == 0), stop=(vt == NVT - 1))
                                fs = fs_p.tile([128, EC], F32, tag="fs")
                                nc.vector.tensor_copy(out=fs, in_=fp)
                                nc.sync.dma_start(
                                    rs_in[128 * tt:128 * (tt + 1),
                                          EC * ei:EC * (ei + 1)], fs)
                        rs_out = dram_rs.tile([RS_ROWS, EGW], F32, tag="rsout")
                        nc.gpsimd.collective_compute(
                            "ReduceScatter", mybir.AluOpType.add,
                            replica_groups=rg,
                            ins=[rs_in.opt()], outs=[rs_out.opt()])
                        fo = fo_p.tile([RS_ROWS, EGW], F32, tag="fo")
                        nc.sync.dma_start(fo, rs_out[:])
                        fd = fo_p.tile([RS_ROWS, EGW], F32, tag="fd")
                        nc.vector.tensor_scalar_mul(
                            out=fd, in0=fo, scalar1=rsg)
                        nc.sync.dma_start(
                            out[sb, :, EGW * eg:EGW * (eg + 1)], fd)

    nc.compile()
    return nc


def _get_nc():
    global _NC_CACHE
    if _NC_CACHE is None:
        _NC_CACHE = build()
    return _NC_CACHE


def _prep_in_maps(vision_feats, W1_w, W1_b, W2_w, llm_token_embed):
    vf = np.ascontiguousarray(np.asarray(vision_feats, np.float32)).reshape(
        T, DV)
    W1 = np.asarray(W1_w, np.float32)
    b1 = np.ascontiguousarray(np.asarray(W1_b, np.float32)).reshape(1, DL)
    W2 = np.asarray(W2_w, np.float32)
    E = np.asarray(llm_token_embed, np.float32)

    w1t = np.ascontiguousarray(W1.T).astype(BF16NP)
    v_loc = 32000 // N_CORES
    in_maps = []
    for c in range(N_CORES):
        vT = np.ascontiguousarray(vf[T_LOC * c:T_LOC * (c + 1)].T).astype(
            BF16NP)
        w2p = np.zeros((V_PAD, DL), np.float32)
        w2p[:v_loc] = W2[v_loc * c:v_loc * (c + 1)]
        # [vt, p, j, vi] with p = d % 128, j = d // 128, vi = v % 128
        w2tt = w2p.T.reshape(NJ, 128, NVT, 128).transpose(2, 1, 0, 3).astype(
            BF16NP)
        ep = np.zeros((V_PAD, DL), np.float32)
        ep[:v_loc] = E[v_loc * c:v_loc * (c + 1)]
        # [e, p, vt, n] with p = v % 128, vt = v // 128, n = d % EC
        ebt = ep.reshape(NVT, 128, N_EC, EC).transpose(2, 1, 0, 3).astype(
            BF16NP)
        in_maps.append({
            "visionT": vT,
            "w1t": w1t,
            "w1b": b1,
            "w2t": np.ascontiguousarray(w2tt),
            "emb": np.ascontiguousarray(ebt),
        })
    return in_maps


def run_on_cores(in_maps, trace=False, **kwargs):
    nc = _get_nc()
    return run_bass_kernel_spmd(nc, in_maps, core_ids=list(range(N_CORES)),
                                trace=trace, **kwargs)


def assemble(core_outs):
    full = np.empty((T, DL), np.float32)
    for c in range(N_CORES):
        o = np.asarray(core_outs[c])  # [N_SB, RS_ROWS, DL]
        for sb in range(N_SB):
            r0 = TSB * sb + RS_ROWS * c
            full[r0:r0 + RS_ROWS] = o[sb]
    return full.reshape(4, 576, DL)


def kernel(**inputs):
    in_maps = _prep_in_maps(**inputs)
    res = run_on_cores(in_maps)
    return assemble([r["out"] for r in res.results])


# revision 5
# speedup vs baseline: 1.0721x; 1.0352x over previous
"""ALIGN module kernel for 8 TRN2 NeuronCores (vocab-parallel).

Reference computation (B=4, S=576, Dv=1024, Dl=4096, V=32000):
    x  = vision_feats @ W1_w.T + W1_b          # [T=2304, Dl]
    xn = layernorm(x)                          # over Dl, no affine
    P  = softmax(xn @ W2_w.T, axis=-1)         # [T, V]
    F  = P @ llm_token_embed                   # [T, Dl]

Sharding: vocab dim of W2_w / llm_token_embed split across the 8 cores
(4000 rows each, zero-padded to 4096). Stage A (W1 + LN) is token-parallel
(288 tokens/core) followed by an AllGather of xn (bf16). Softmax needs no
max-subtraction (logits are ~N(0,1), |logit| < ~6): each core computes
exp(logits_loc); the 96 zero pad rows contribute exactly exp(0)=1 each, so
the local denominator is corrected by -96. Denominators are ReduceScattered
(each core only ever divides the 96-token slices it owns after the F
ReduceScatter, so no cross-core broadcast of s is needed at all).

Host-side prep encodes all layout work: weights arrive pre-transposed,
pre-padded, pre-tiled for unit-stride DMA, and pre-cast to bf16, so the
device never transposes W1/W2 and every big DMA runs at full burst size.
"""

import os
import sys

for _p in ("/opt/trn_rl_repo", "/root/.axon_site/_ro/trn_rl_repo"):
    if os.path.isdir(_p) and _p not in sys.path:
        sys.path.insert(0, _p)

import numpy as np
import ml_dtypes

from concourse import bass, bacc, mybir, tile
from concourse.bass_utils import run_bass_kernel_spmd
from concourse.masks import make_identity

BF16NP = ml_dtypes.bfloat16
F32 = mybir.dt.float32
BF16 = mybir.dt.bfloat16

N_CORES = 8
T = 2304          # total tokens (B*S)
T_LOC = 288       # tokens per core in stage A
DV = 1024
DL = 4096
V_PAD = 4096      # padded vocab rows per core (4000 real + 96 zero pads)
N_PAD = 96.0
NVT = V_PAD // 128  # 32 vocab tiles per core
NJ = DL // 128      # 32 contraction tiles

N_SB = 3          # token superblocks in phase B
TSB = 768         # tokens per superblock
N_TT = 6          # 128-row token tiles per superblock
C1 = 384          # matmul1 token-chunk width (2 chunks per superblock)
EC = 256          # matmul2 embedding-chunk width (SBUF tile)
N_EC = DL // EC   # 16 e-chunks
EG = 4            # e-chunks per ReduceScatter group
N_EG = N_EC // EG
EGW = EC * EG     # 1024 columns per RS
RS_ROWS = TSB // N_CORES  # 96 rows per core out of each ReduceScatter

_NC_CACHE = None


def build():
    nc = bacc.Bacc("TRN2", target_bir_lowering=False, debug=False,
                   num_devices=N_CORES)
    rg = [list(range(N_CORES))]

    visionT = nc.dram_tensor("visionT", [DV, T_LOC], BF16, kind="ExternalInput")
    w1t = nc.dram_tensor("w1t", [DV, DL], BF16, kind="ExternalInput")
    w1b = nc.dram_tensor("w1b", [1, DL], F32, kind="ExternalInput")
    # [vt][p][j][vi]: per-partition unit-stride 8KB runs
    w2t = nc.dram_tensor("w2t", [NVT, 128, NJ, 128], BF16, kind="ExternalInput")
    # [e][p][vt][n]: per-partition unit-stride 16KB runs
    emb = nc.dram_tensor("emb", [N_EC, 128, NVT, EC], BF16,
                         kind="ExternalInput")
    ones_v = nc.dram_tensor("ones_v", [128, NVT, 1], BF16, kind="ExternalInput")
    out = nc.dram_tensor("out", [N_SB, RS_ROWS, DL], F32, kind="ExternalOutput")

    from contextlib import ExitStack
    with tile.TileContext(nc) as tc, ExitStack() as ctx:
        consts = ctx.enter_context(tc.tile_pool(name="consts", bufs=1))
        dram = ctx.enter_context(tc.tile_pool(name="dram", bufs=1, space="DRAM"))
        dram_s = ctx.enter_context(tc.tile_pool(name="dram_s", bufs=2, space="DRAM"))
        dram_rs = ctx.enter_context(tc.tile_pool(name="dram_rs", bufs=3, space="DRAM"))
        if True:

            ident = consts.tile([128, 128], BF16)
            make_identity(nc, ident)
            onesv_sb = consts.tile([128, NVT, 1], BF16)
            nc.sync.dma_start(onesv_sb, ones_v[:])
            eps_col = consts.tile([128, 1], F32)
            nc.vector.memset(eps_col, 1e-5)

            ag_in = dram.tile([T_LOC, DL], BF16)
            ag_out = dram.tile([T, DL], BF16, addr_space="Shared")

            # ---------------- Stage A: x = visionT.T @ W1T + b, LN, -> bf16
            with ExitStack() as actx:
                sa = actx.enter_context(tc.tile_pool(name="stageA", bufs=1))
                sa2 = actx.enter_context(tc.tile_pool(name="stageA2", bufs=2))
                psa = actx.enter_context(tc.tile_pool(name="psumA", bufs=2, space="PSUM"))
                vt_sb = sa.tile([128, DV // 128, T_LOC], BF16)
                for k in range(DV // 128):
                    nc.sync.dma_start(
                        vt_sb[:, k, :], visionT[128 * k:128 * (k + 1), :])
                w1t_sb = sa.tile([128, DV // 128, DL], BF16)
                for k in range(DV // 128):
                    nc.sync.dma_start(
                        w1t_sb[:, k, :], w1t[128 * k:128 * (k + 1), :])
                bias_bc = sa.tile([128, DL], F32)
                nc.sync.dma_start(
                    bias_bc,
                    bass.AP(tensor=w1b, offset=0, ap=[[0, 128], [1, DL]]))

                t_sizes = [128, 128, 32]
                for a in range(3):
                    ta = t_sizes[a]
                    t0 = 128 * a
                    x_sb = sa2.tile([128, DL], F32, tag="x")
                    for n in range(DL // 512):
                        xp = psa.tile([128, 512], F32, tag="xp")
                        for k in range(DV // 128):
                            nc.tensor.matmul(
                                xp[:ta], lhsT=vt_sb[:, k, t0:t0 + ta],
                                rhs=w1t_sb[:, k, 512 * n:512 * (n + 1)],
                                start=(k == 0), stop=(k == DV // 128 - 1))
                        nc.vector.tensor_tensor(
                            out=x_sb[:ta, 512 * n:512 * (n + 1)],
                            in0=xp[:ta],
                            in1=bias_bc[:ta, 512 * n:512 * (n + 1)],
                            op=mybir.AluOpType.add)
                    # LayerNorm over DL
                    stats = sa2.tile([128, DL // 512, 6], F32, tag="stats")
                    for g in range(DL // 512):
                        nc.vector.bn_stats(
                            out=stats[:ta, g, :],
                            in_=x_sb[:ta, 512 * g:512 * (g + 1)])
                    mv = sa2.tile([128, 2], F32, tag="mv")
                    nc.vector.bn_aggr(out=mv[:ta], in_=stats[:ta])
                    sd = sa2.tile([128, 1], F32, tag="sd")
                    nc.scalar.activation(
                        out=sd[:ta], in_=mv[:ta, 1:2],
                        func=mybir.ActivationFunctionType.Sqrt,
                        bias=eps_col[:ta])
                    rstd = sa2.tile([128, 1], F32, tag="rstd")
                    nc.vector.reciprocal(out=rstd[:ta], in_=sd[:ta])
                    xn_bf = sa2.tile([128, DL], BF16, tag="xn")
                    nc.vector.tensor_scalar(
                        out=xn_bf[:ta], in0=x_sb[:ta],
                        scalar1=mv[:ta, 0:1], scalar2=rstd[:ta],
                        op0=mybir.AluOpType.subtract,
                        op1=mybir.AluOpType.mult)
                    nc.sync.dma_start(ag_in[t0:t0 + ta, :], xn_bf[:ta])

            nc.gpsimd.collective_compute(
                "AllGather", mybir.AluOpType.bypass, replica_groups=rg,
                ins=[ag_in.opt()], outs=[ag_out.opt()])

            # ---------------- Phase B
            xnt_p = ctx.enter_context(tc.tile_pool(name="xnt_p", bufs=1))
            pt_p = ctx.enter_context(tc.tile_pool(name="pt_p", bufs=1))
            xl_p = ctx.enter_context(tc.tile_pool(name="xl_p", bufs=2))
            w2_p = ctx.enter_context(tc.tile_pool(name="w2_p", bufs=3))
            eb_p = ctx.enter_context(tc.tile_pool(name="eb_p", bufs=2))
            fs_p = ctx.enter_context(tc.tile_pool(name="fs_p", bufs=3))
            fo_p = ctx.enter_context(tc.tile_pool(name="fo_p", bufs=2))
            small = ctx.enter_context(tc.tile_pool(name="small", bufs=2))
            tp_ps = ctx.enter_context(tc.tile_pool(name="tp_ps", bufs=2, space="PSUM"))
            l_ps = ctx.enter_context(tc.tile_pool(name="l_ps", bufs=2, space="PSUM"))
            f_ps = ctx.enter_context(tc.tile_pool(name="f_ps", bufs=4, space="PSUM"))
            if True:

                for sb in range(N_SB):
                    # transpose xn superblock -> xnt [d_local, j, t_local]
                    xnt = xnt_p.tile([128, NJ, TSB], BF16, tag="xnt")
                    for tt in range(N_TT):
                        xl = xl_p.tile([128, DL], BF16, tag="xl")
                        r0 = TSB * sb + 128 * tt
                        nc.sync.dma_start(xl, ag_out[r0:r0 + 128, :])
                        for j in range(NJ):
                            tp = tp_ps.tile([128, 128], BF16, tag="tp")
                            nc.tensor.transpose(
                                out=tp, in_=xl[:, 128 * j:128 * (j + 1)],
                                identity=ident)
                            nc.vector.tensor_copy(
                                out=xnt[:, j, 128 * tt:128 * (tt + 1)],
                                in_=tp)

                    # matmul1: logitsT per v-tile, exp -> pt
                    pt = pt_p.tile([128, NVT, TSB], BF16, tag="pt")
                    for vt in range(NVT):
                        w2s = w2_p.tile([128, NJ, 128], BF16, tag="w2")
                        nc.sync.dma_start(w2s, w2t[vt])
                        for c in range(2):
                            lp = l_ps.tile([128, C1], F32, tag="lp")
                            for j in range(NJ):
                                nc.tensor.matmul(
                                    lp, lhsT=w2s[:, j, :],
                                    rhs=xnt[:, j, C1 * c:C1 * (c + 1)],
                                    start=(j == 0), stop=(j == NJ - 1))
                            nc.scalar.activation(
                                out=pt[:, vt, C1 * c:C1 * (c + 1)], in_=lp,
                                func=mybir.ActivationFunctionType.Exp)

                    # matmul2: F_partial = pt.T @ emb, RS per e-group.
                    # e-chunk 0 carries an extra ones-masked column that
                    # accumulates the softmax denominator (pads excluded),
                    # so s rides the same ReduceScatter as the numerator;
                    # division happens locally on owned rows after each RS.
                    # Last group of the last superblock is split small to
                    # shrink the exposed tail.
                    egroups = [4, 4, 4, 4] if sb < N_SB - 1 else [4, 4, 4, 3, 1]
                    rsg = None
                    e = 0
                    col = 0
                    for gi, gsz in enumerate(egroups):
                        gw = gsz * EC + (1 if gi == 0 else 0)
                        rs_in = dram_rs.tile([TSB, gw], F32, tag="rsin",
                                             name=f"rsin_{sb}_{gi}")
                        for ei in range(gsz):
                            aug = 1 if (gi == 0 and ei == 0) else 0
                            ew = EC + aug
                            eb = eb_p.tile([128, NVT, EC + 1], BF16, tag="eb")
                            nc.sync.dma_start(eb[:, :, :EC], emb[e])
                            if aug:
                                nc.vector.tensor_copy(
                                    out=eb[:, :, EC:EC + 1], in_=onesv_sb)
                            for tt in range(N_TT):
                                fp = f_ps.tile([128, EC + 1], F32, tag="fp")
                                for vt in range(NVT):
                                    nc.tensor.matmul(
                                        fp[:, :ew],
                                        lhsT=pt[:, vt, 128 * tt:128 * (tt + 1)],
                                        rhs=eb[:, vt, :ew],
                                        start=(vt == 0), stop=(vt == NVT - 1))
                                fs = fs_p.tile([128, EC + 1], F32, tag="fs")
                                nc.vector.tensor_copy(
                                    out=fs[:, :ew], in_=fp[:, :ew])
                                nc.sync.dma_start(
                                    rs_in[128 * tt:128 * (tt + 1),
                                          EC * ei:EC * ei + EC], fs[:, :EC])
                                if aug:
                                    nc.sync.dma_start(
                                        rs_in[128 * tt:128 * (tt + 1),
                                              gw - 1:gw], fs[:, EC:EC + 1])
                            e += 1
                        rs_out = dram_rs.tile([RS_ROWS, gw], F32, tag="rsout",
                                              name=f"rsout_{sb}_{gi}")
                        nc.gpsimd.collective_compute(
                            "ReduceScatter", mybir.AluOpType.add,
                            replica_groups=rg,
                            ins=[rs_in.opt()], outs=[rs_out.opt()])
                        fo = fo_p.tile([RS_ROWS, EC * EG + 1], F32, tag="fo",
                                       name=f"fo_{sb}_{gi}")
                        nc.sync.dma_start(fo[:, :gw], rs_out[:])
                        if gi == 0:
                            rsg = small.tile([RS_ROWS, 1], F32, tag="rsg")
                            nc.vector.reciprocal(
                                out=rsg, in_=fo[:, gw - 1:gw])
                        fd = fo_p.tile([RS_ROWS, EC * EG], F32, tag="fd",
                                       name=f"fd_{sb}_{gi}")
                        fw = gsz * EC
                        nc.vector.tensor_scalar_mul(
                            out=fd[:, :fw], in0=fo[:, :fw], scalar1=rsg)
                        nc.sync.dma_start(
                            out[sb, :, col:col + fw], fd[:, :fw])
                        col += fw

    nc.compile()
    return nc


def _get_nc():
    global _NC_CACHE
    if _NC_CACHE is None:
        _NC_CACHE = build()
    return _NC_CACHE


def _prep_in_maps(vision_feats, W1_w, W1_b, W2_w, llm_token_embed):
    vf = np.ascontiguousarray(np.asarray(vision_feats, np.float32)).reshape(
        T, DV)
    W1 = np.asarray(W1_w, np.float32)
    b1 = np.ascontiguousarray(np.asarray(W1_b, np.float32)).reshape(1, DL)
    W2 = np.asarray(W2_w, np.float32)
    E = np.asarray(llm_token_embed, np.float32)

    w1t = np.ascontiguousarray(W1.T).astype(BF16NP)
    v_loc = 32000 // N_CORES
    in_maps = []
    for c in range(N_CORES):
        vT = np.ascontiguousarray(vf[T_LOC * c:T_LOC * (c + 1)].T).astype(
            BF16NP)
        w2p = np.zeros((V_PAD, DL), np.float32)
        w2p[:v_loc] = W2[v_loc * c:v_loc * (c + 1)]
        # [vt, p, j, vi] with p = d % 128, j = d // 128, vi = v % 128
        w2tt = w2p.T.reshape(NJ, 128, NVT, 128).transpose(2, 1, 0, 3).astype(
            BF16NP)
        ep = np.zeros((V_PAD, DL), np.float32)
        ep[:v_loc] = E[v_loc * c:v_loc * (c + 1)]
        # [e, p, vt, n] with p = v % 128, vt = v // 128, n = d % EC
        ebt = ep.reshape(NVT, 128, N_EC, EC).transpose(2, 1, 0, 3).astype(
            BF16NP)
        onesv = np.zeros((128, NVT, 1), np.float32)
        for vt in range(NVT):
            for p in range(128):
                if 128 * vt + p < v_loc:
                    onesv[p, vt, 0] = 1.0
        in_maps.append({
            "visionT": vT,
            "w1t": w1t,
            "w1b": b1,
            "w2t": np.ascontiguousarray(w2tt),
            "emb": np.ascontiguousarray(ebt),
            "ones_v": onesv.astype(BF16NP),
        })
    return in_maps


def run_on_cores(in_maps, trace=False, **kwargs):
    nc = _get_nc()
    return run_bass_kernel_spmd(nc, in_maps, core_ids=list(range(N_CORES)),
                                trace=trace, **kwargs)


def assemble(core_outs):
    full = np.empty((T, DL), np.float32)
    for c in range(N_CORES):
        o = np.asarray(core_outs[c])  # [N_SB, RS_ROWS, DL]
        for sb in range(N_SB):
            r0 = TSB * sb + RS_ROWS * c
            full[r0:r0 + RS_ROWS] = o[sb]
    return full.reshape(4, 576, DL)


def kernel(**inputs):
    in_maps = _prep_in_maps(**inputs)
    res = run_on_cores(in_maps)
    return assemble([r["out"] for r in res.results])


# revision 8
# speedup vs baseline: 1.0804x; 1.0077x over previous
"""ALIGN module kernel for 8 TRN2 NeuronCores (vocab-parallel).

Reference computation (B=4, S=576, Dv=1024, Dl=4096, V=32000):
    x  = vision_feats @ W1_w.T + W1_b          # [T=2304, Dl]
    xn = layernorm(x)                          # over Dl, no affine
    P  = softmax(xn @ W2_w.T, axis=-1)         # [T, V]
    F  = P @ llm_token_embed                   # [T, Dl]

Sharding: vocab dim of W2_w / llm_token_embed split across the 8 cores
(4000 rows each, zero-padded to 4096). Stage A (W1 + LN) is token-parallel
(288 tokens/core) followed by an AllGather of xn (bf16). Softmax needs no
max-subtraction (logits are ~N(0,1), |logit| < ~6): each core computes
exp(logits_loc); the 96 zero pad rows contribute exactly exp(0)=1 each, so
the local denominator is corrected by -96. Denominators are ReduceScattered
(each core only ever divides the 96-token slices it owns after the F
ReduceScatter, so no cross-core broadcast of s is needed at all).

Host-side prep encodes all layout work: weights arrive pre-transposed,
pre-padded, pre-tiled for unit-stride DMA, and pre-cast to bf16, so the
device never transposes W1/W2 and every big DMA runs at full burst size.
"""

import os
import sys

for _p in ("/opt/trn_rl_repo", "/root/.axon_site/_ro/trn_rl_repo"):
    if os.path.isdir(_p) and _p not in sys.path:
        sys.path.insert(0, _p)

import numpy as np
import ml_dtypes

from concourse import bass, bacc, mybir, tile
from concourse.bass_utils import run_bass_kernel_spmd
from concourse.masks import make_identity

BF16NP = ml_dtypes.bfloat16
F32 = mybir.dt.float32
BF16 = mybir.dt.bfloat16

N_CORES = 8
T = 2304          # total tokens (B*S)
T_LOC = 288       # tokens per core in stage A
DV = 1024
DL = 4096
V_PAD = 4096      # padded vocab rows per core (4000 real + 96 zero pads)
N_PAD = 96.0
NVT = V_PAD // 128  # 32 vocab tiles per core
NJ = DL // 128      # 32 contraction tiles

N_SB = 3          # token superblocks in phase B
TSB = 768         # tokens per superblock
N_TT = 6          # 128-row token tiles per superblock
C1 = 384          # matmul1 token-chunk width (2 chunks per superblock)
EC = 512          # matmul2 embedding-chunk width (SBUF tile)
N_EC = DL // EC   # 8 e-chunks
EG = 2            # e-chunks per ReduceScatter group
N_EG = N_EC // EG
EGW = EC * EG     # 1024 columns per RS
RS_ROWS = TSB // N_CORES  # 96 rows per core out of each ReduceScatter

_NC_CACHE = None


def build():
    nc = bacc.Bacc("TRN2", target_bir_lowering=False, debug=False,
                   num_devices=N_CORES)
    rg = [list(range(N_CORES))]

    visionT = nc.dram_tensor("visionT", [DV, T_LOC], BF16, kind="ExternalInput")
    w1t = nc.dram_tensor("w1t", [DV, DL], BF16, kind="ExternalInput")
    w1b = nc.dram_tensor("w1b", [1, DL], F32, kind="ExternalInput")
    # [vt][p][j][vi]: per-partition unit-stride 8KB runs
    w2t = nc.dram_tensor("w2t", [NVT, 128, NJ, 128], BF16, kind="ExternalInput")
    # [e][p][vt][n]: per-partition unit-stride 16KB runs
    emb = nc.dram_tensor("emb", [N_EC, 128, NVT, EC], BF16,
                         kind="ExternalInput")
    ones_v = nc.dram_tensor("ones_v", [128, NVT, 1], BF16, kind="ExternalInput")
    out = nc.dram_tensor("out", [N_SB, RS_ROWS, DL], F32, kind="ExternalOutput")

    from contextlib import ExitStack
    with tile.TileContext(nc) as tc, ExitStack() as ctx:
        consts = ctx.enter_context(tc.tile_pool(name="consts", bufs=1))
        dram = ctx.enter_context(tc.tile_pool(name="dram", bufs=1, space="DRAM"))
        dram_s = ctx.enter_context(tc.tile_pool(name="dram_s", bufs=2, space="DRAM"))
        dram_rs = ctx.enter_context(tc.tile_pool(name="dram_rs", bufs=3, space="DRAM"))
        if True:

            ident = consts.tile([128, 128], BF16)
            make_identity(nc, ident)
            onesv_sb = consts.tile([128, NVT, 1], BF16)
            nc.sync.dma_start(onesv_sb, ones_v[:])
            eps_col = consts.tile([128, 1], F32)
            nc.vector.memset(eps_col, 1e-5)

            ag_in = dram.tile([T_LOC, DL], BF16)
            ag_out = dram.tile([T, DL], BF16, addr_space="Shared")

            # ---------------- Stage A: x = visionT.T @ W1T + b, LN, -> bf16
            with ExitStack() as actx:
                sa = actx.enter_context(tc.tile_pool(name="stageA", bufs=1))
                sa2 = actx.enter_context(tc.tile_pool(name="stageA2", bufs=2))
                psa = actx.enter_context(tc.tile_pool(name="psumA", bufs=2, space="PSUM"))
                vt_sb = sa.tile([128, DV // 128, T_LOC], BF16)
                for k in range(DV // 128):
                    nc.sync.dma_start(
                        vt_sb[:, k, :], visionT[128 * k:128 * (k + 1), :])
                w1t_sb = sa.tile([128, DV // 128, DL], BF16)
                for k in range(DV // 128):
                    nc.sync.dma_start(
                        w1t_sb[:, k, :], w1t[128 * k:128 * (k + 1), :])
                bias_bc = sa.tile([128, DL], F32)
                nc.sync.dma_start(
                    bias_bc,
                    bass.AP(tensor=w1b, offset=0, ap=[[0, 128], [1, DL]]))

                t_sizes = [128, 128, 32]
                for a in range(3):
                    ta = t_sizes[a]
                    t0 = 128 * a
                    x_sb = sa2.tile([128, DL], F32, tag="x")
                    for n in range(DL // 512):
                        xp = psa.tile([128, 512], F32, tag="xp")
                        for k in range(DV // 128):
                            nc.tensor.matmul(
                                xp[:ta], lhsT=vt_sb[:, k, t0:t0 + ta],
                                rhs=w1t_sb[:, k, 512 * n:512 * (n + 1)],
                                start=(k == 0), stop=(k == DV // 128 - 1))
                        nc.vector.tensor_tensor(
                            out=x_sb[:ta, 512 * n:512 * (n + 1)],
                            in0=xp[:ta],
                            in1=bias_bc[:ta, 512 * n:512 * (n + 1)],
                            op=mybir.AluOpType.add)
                    # LayerNorm over DL
                    stats = sa2.tile([128, DL // 512, 6], F32, tag="stats")
                    for g in range(DL // 512):
                        nc.vector.bn_stats(
                            out=stats[:ta, g, :],
                            in_=x_sb[:ta, 512 * g:512 * (g + 1)])
                    mv = sa2.tile([128, 2], F32, tag="mv")
                    nc.vector.bn_aggr(out=mv[:ta], in_=stats[:ta])
                    sd = sa2.tile([128, 1], F32, tag="sd")
                    nc.scalar.activation(
                        out=sd[:ta], in_=mv[:ta, 1:2],
                        func=mybir.ActivationFunctionType.Sqrt,
                        bias=eps_col[:ta])
                    rstd = sa2.tile([128, 1], F32, tag="rstd")
                    nc.vector.reciprocal(out=rstd[:ta], in_=sd[:ta])
                    xn_bf = sa2.tile([128, DL], BF16, tag="xn")
                    nc.vector.tensor_scalar(
                        out=xn_bf[:ta], in0=x_sb[:ta],
                        scalar1=mv[:ta, 0:1], scalar2=rstd[:ta],
                        op0=mybir.AluOpType.subtract,
                        op1=mybir.AluOpType.mult)
                    nc.sync.dma_start(ag_in[t0:t0 + ta, :], xn_bf[:ta])

            nc.gpsimd.collective_compute(
                "AllGather", mybir.AluOpType.bypass, replica_groups=rg,
                ins=[ag_in.opt()], outs=[ag_out.opt()])

            # ---------------- Phase B
            xnt_p = ctx.enter_context(tc.tile_pool(name="xnt_p", bufs=1))
            pt_p = ctx.enter_context(tc.tile_pool(name="pt_p", bufs=1))
            xl_p = ctx.enter_context(tc.tile_pool(name="xl_p", bufs=2))
            w2_p = ctx.enter_context(tc.tile_pool(name="w2_p", bufs=2))
            eb_p = ctx.enter_context(tc.tile_pool(name="eb_p", bufs=2))
            fs_p = ctx.enter_context(tc.tile_pool(name="fs_p", bufs=2))
            fo_p = ctx.enter_context(tc.tile_pool(name="fo_p", bufs=1))
            small = ctx.enter_context(tc.tile_pool(name="small", bufs=2))
            tp_ps = ctx.enter_context(tc.tile_pool(name="tp_ps", bufs=2, space="PSUM"))
            l_ps = ctx.enter_context(tc.tile_pool(name="l_ps", bufs=2, space="PSUM"))
            s_ps = ctx.enter_context(tc.tile_pool(name="s_ps", bufs=1, space="PSUM"))
            f_ps = ctx.enter_context(tc.tile_pool(name="f_ps", bufs=2, space="PSUM"))
            if True:

                for sb in range(N_SB):
                    # transpose xn superblock -> xnt [d_local, j, t_local]
                    xnt = xnt_p.tile([128, NJ, TSB], BF16, tag="xnt")
                    for tt in range(N_TT):
                        xl = xl_p.tile([128, DL], BF16, tag="xl")
                        r0 = TSB * sb + 128 * tt
                        nc.sync.dma_start(xl, ag_out[r0:r0 + 128, :])
                        for j in range(NJ):
                            tp = tp_ps.tile([128, 128], BF16, tag="tp")
                            nc.tensor.transpose(
                                out=tp, in_=xl[:, 128 * j:128 * (j + 1)],
                                identity=ident)
                            nc.vector.tensor_copy(
                                out=xnt[:, j, 128 * tt:128 * (tt + 1)],
                                in_=tp)

                    # matmul1: logitsT per v-tile, exp -> pt
                    pt = pt_p.tile([128, NVT, TSB], BF16, tag="pt")
                    for vt in range(NVT):
                        w2s = w2_p.tile([128, NJ, 128], BF16, tag="w2")
                        nc.sync.dma_start(w2s, w2t[vt])
                        for c in range(2):
                            lp = l_ps.tile([128, C1], F32, tag="lp")
                            for j in range(NJ):
                                nc.tensor.matmul(
                                    lp, lhsT=w2s[:, j, :],
                                    rhs=xnt[:, j, C1 * c:C1 * (c + 1)],
                                    start=(j == 0), stop=(j == NJ - 1))
                            nc.scalar.activation(
                                out=pt[:, vt, C1 * c:C1 * (c + 1)], in_=lp,
                                func=mybir.ActivationFunctionType.Exp)

                    # denominator: s[t] = sum over real v rows of pt
                    # (onesv masks out the 96 zero-pad rows), then a tiny
                    # ReduceScatter hands each core exactly the 96-token
                    # slice it will own after the F ReduceScatters.
                    swidths = [(0, 512), (512, 256)]
                    sps = []
                    for c, (s0, sw) in enumerate(swidths):
                        sp = s_ps.tile([1, 512], F32, tag=f"sp{c}")
                        for vt in range(NVT):
                            nc.tensor.matmul(
                                sp[:, :sw], lhsT=onesv_sb[:, vt, :],
                                rhs=pt[:, vt, s0:s0 + sw],
                                start=(vt == 0), stop=(vt == NVT - 1))
                        sps.append(sp)
                    s_sb = small.tile([1, TSB], F32, tag="ssb", bufs=1)
                    for c, (s0, sw) in enumerate(swidths):
                        nc.vector.tensor_copy(
                            out=s_sb[0:1, s0:s0 + sw], in_=sps[c][:, :sw])
                    s_in = dram_s.tile([1, TSB], F32, tag="sin")
                    nc.sync.dma_start(s_in, s_sb)
                    s_out = dram_s.tile([1, RS_ROWS], F32, tag="sout")
                    nc.gpsimd.collective_compute(
                        "ReduceScatter", mybir.AluOpType.add, replica_groups=rg,
                        ins=[s_in.opt()], outs=[s_out.opt()])
                    sg = small.tile([RS_ROWS, 1], F32, tag="sg")
                    nc.sync.dma_start(
                        sg,
                        bass.AP(tensor=s_out.tensor, offset=s_out.offset,
                                ap=[[1, RS_ROWS], [1, 1]]))
                    rsg = small.tile([RS_ROWS, 1], F32, tag="rsg")
                    nc.vector.reciprocal(out=rsg, in_=sg)

                    # matmul2: F_partial = pt.T @ emb, RS per e-group, local
                    # divide on owned rows. Last superblock splits its final
                    # group to shrink the exposed RS tail.
                    egroups = [2, 2, 2, 2] if sb < N_SB - 1 else [2, 2, 2, 1, 1]
                    col = 0
                    e = 0
                    for gi, gsz in enumerate(egroups):
                        gw = gsz * EC
                        rs_in = dram_rs.tile([TSB, gw], F32, tag="rsin",
                                             name=f"rsin_{sb}_{gi}")
                        for ei in range(gsz):
                            eb = eb_p.tile([128, NVT, EC], BF16, tag="eb")
                            nc.sync.dma_start(eb, emb[e])
                            for tt in range(N_TT):
                                fp = f_ps.tile([128, EC], F32, tag="fp")
                                for vt in range(NVT):
                                    nc.tensor.matmul(
                                        fp,
                                        lhsT=pt[:, vt, 128 * tt:128 * (tt + 1)],
                                        rhs=eb[:, vt, :],
                                        start=(vt == 0), stop=(vt == NVT - 1))
                                fs = fs_p.tile([128, EC], F32, tag="fs")
                                nc.vector.tensor_copy(out=fs, in_=fp)
                                nc.sync.dma_start(
                                    rs_in[128 * tt:128 * (tt + 1),
                                          EC * ei:EC * (ei + 1)], fs)
                            e += 1
                        rs_out = dram_rs.tile([RS_ROWS, gw], F32, tag="rsout",
                                              name=f"rsout_{sb}_{gi}")
                        nc.gpsimd.collective_compute(
                            "ReduceScatter", mybir.AluOpType.add,
                            replica_groups=rg,
                            ins=[rs_in.opt()], outs=[rs_out.opt()])
                        fo = fo_p.tile([RS_ROWS, EGW], F32, tag="fo",
                                       name=f"fo_{sb}_{gi}")
                        nc.sync.dma_start(fo[:, :gw], rs_out[:])
                        fd = fo_p.tile([RS_ROWS, EGW], F32, tag="fd",
                                       name=f"fd_{sb}_{gi}")
                        nc.vector.tensor_scalar_mul(
                            out=fd[:, :gw], in0=fo[:, :gw], scalar1=rsg)
                        nc.sync.dma_start(
                            out[sb, :, col:col + gw], fd[:, :gw])
                        col += gw

    nc.compile()
    return nc


def _get_nc():
    global _NC_CACHE
    if _NC_CACHE is None:
        _NC_CACHE = build()
    return _NC_CACHE


def _prep_in_maps(vision_feats, W1_w, W1_b, W2_w, llm_token_embed):
    vf = np.ascontiguousarray(np.asarray(vision_feats, np.float32)).reshape(
        T, DV)
    W1 = np.asarray(W1_w, np.float32)
    b1 = np.ascontiguousarray(np.asarray(W1_b, np.float32)).reshape(1, DL)
    W2 = np.asarray(W2_w, np.float32)
    E = np.asarray(llm_token_embed, np.float32)

    w1t = np.ascontiguousarray(W1.T).astype(BF16NP)
    v_loc = 32000 // N_CORES
    in_maps = []
    for c in range(N_CORES):
        vT = np.ascontiguousarray(vf[T_LOC * c:T_LOC * (c + 1)].T).astype(
            BF16NP)
        w2p = np.zeros((V_PAD, DL), np.float32)
        w2p[:v_loc] = W2[v_loc * c:v_loc * (c + 1)]
        # [vt, p, j, vi] with p = d % 128, j = d // 128, vi = v % 128
        w2tt = w2p.T.reshape(NJ, 128, NVT, 128).transpose(2, 1, 0, 3).astype(
            BF16NP)
        ep = np.zeros((V_PAD, DL), np.float32)
        ep[:v_loc] = E[v_loc * c:v_loc * (c + 1)]
        # [e, p, vt, n] with p = v % 128, vt = v // 128, n = d % EC
        ebt = ep.reshape(NVT, 128, N_EC, EC).transpose(2, 1, 0, 3).astype(
            BF16NP)
        onesv = np.zeros((128, NVT, 1), np.float32)
        for vt in range(NVT):
            for p in range(128):
                if 128 * vt + p < v_loc:
                    onesv[p, vt, 0] = 1.0
        in_maps.append({
            "visionT": vT,
            "w1t": w1t,
            "w1b": b1,
            "w2t": np.ascontiguousarray(w2tt),
            "emb": np.ascontiguousarray(ebt),
            "ones_v": onesv.astype(BF16NP),
        })
    return in_maps


def run_on_cores(in_maps, trace=False, **kwargs):
    nc = _get_nc()
    return run_bass_kernel_spmd(nc, in_maps, core_ids=list(range(N_CORES)),
                                trace=trace, **kwargs)


def assemble(core_outs):
    full = np.empty((T, DL), np.float32)
    for c in range(N_CORES):
        o = np.asarray(core_outs[c])  # [N_SB, RS_ROWS, DL]
        for sb in range(N_SB):
            r0 = TSB * sb + RS_ROWS * c
            full[r0:r0 + RS_ROWS] = o[sb]
    return full.reshape(4, 576, DL)


def kernel(**inputs):
    in_maps = _prep_in_maps(**inputs)
    res = run_on_cores(in_maps)
    return assemble([r["out"] for r in res.results])
